# revision 6
# baseline (speedup 1.0000x reference)
"""GAT(4 heads, concat) -> GCN fused kernel for trn2 x 8 NeuronCores.

Sharding: edges sharded by dst range (8 contiguous node ranges, one per
core). Each core owns its dst-range rows of every intermediate, so the
segment softmax and both segment sums are core-local. Per-edge source
rows are fetched with indirect (descriptor-generated) DMA gathers from a
replicated node table; the segment sums are done as one-hot scatter
matmuls on the tensor engine accumulating in PSUM.

Two NEFF dispatches:
  NEFF1: xp = x@W_gat (+ per-node attention logits) -> per-edge GAT
         softmax/aggregate -> y' rows (= dinv * y_gat) for own dst range.
  host:  concat y' slices from the 8 cores (pure resharding glue).
  NEFF2: z = sum_e ew * y'[src] per dst, out = (dinv*z) @ W_gcn.
"""
import sys, types, math

sys.path.insert(0, "/opt/trn_rl_repo")
import numpy as np
import ml_dtypes

# antenv.axon_hooks is absent from this image; shim it so
# run_bass_kernel_spmd(trace=True) can register the NTFF profile hook.
if "antenv.axon_hooks" not in sys.modules:
    _m = types.ModuleType("antenv.axon_hooks")
    _box = [None]
    _m.set_axon_ntff_profile_hook = lambda h: _box.__setitem__(0, h)
    _m.get_axon_ntff_profile_hook = lambda: _box[0]
    sys.modules["antenv.axon_hooks"] = _m
    try:
        from trn_agent_boot.trn_boot import _ntff_profile_via_ctypes
        _m.set_axon_ntff_profile_hook(
            _ntff_profile_via_ctypes("/opt/axon/libaxon_pjrt.so"))
    except Exception:
        pass

import concourse.bass as bass
import concourse.tile as tile
from concourse import bacc, mybir
from concourse.bass_utils import run_bass_kernel_spmd

F32 = mybir.dt.float32
BF16 = mybir.dt.bfloat16
I32 = mybir.dt.int32

N_CORES = 8
P = 128
NEG_SLOPE = 0.2
LAST_EXEC_NS = [None, None]  # exec_time_ns of the two dispatches (for test harness)
DEBUG = {}


def _prep(x, edge_index, edge_weight):
    N, IN = x.shape
    E = edge_index.shape[1]
    src = np.concatenate([edge_index[0], np.arange(N, dtype=np.int64)])
    dst = np.concatenate([edge_index[1], np.arange(N, dtype=np.int64)])
    ew = np.concatenate([edge_weight.astype(np.float32), np.ones(N, np.float32)])

    NT = math.ceil(N / (P * N_CORES))     # dst tiles per core
    NP = NT * P * N_CORES                 # padded node count
    span = NT * P                         # dst range per core

    core_of = dst // span
    tau_of = (dst % span) // P
    # stable ordering by (core, tau)
    order = np.lexsort((tau_of, core_of))
    src, dst, ew = src[order], dst[order], ew[order]
    core_of, tau_of = core_of[order], tau_of[order]

    # counts[c, tau]
    counts = np.zeros((N_CORES, NT), np.int64)
    np.add.at(counts, (core_of, tau_of), 1)
    m_tiles = np.ceil(counts.max(axis=0) / P).astype(np.int64)  # per-tau tile count
    T = int(m_tiles.sum())
    col0 = np.concatenate([[0], np.cumsum(m_tiles)])[:-1]       # stream col base per tau

    src_s = np.zeros((N_CORES, P, T), np.int32)
    dst_s = np.full((N_CORES, P, T), 200.0, np.float32)          # sentinel -> no one-hot match
    ew_s = np.zeros((N_CORES, P, T), np.float32)
    dstg_s = np.zeros((N_CORES, P, NT), np.int32)                # global dst ids per tau

    # slot positions for every edge without a python loop over edges:
    # rank of edge within its (core,tau) bucket
    start_of = np.zeros((N_CORES, NT), np.int64)
    flat = core_of * NT + tau_of
    # edges already sorted by flat; rank = index - first occurrence
    first = np.concatenate([[0], np.cumsum(np.bincount(flat, minlength=N_CORES * NT))])[:-1]
    rank = np.arange(len(src)) - first[flat]
    col = col0[tau_of] + rank // P
    row = rank % P
    src_s[core_of, row, col] = src
    dst_s[core_of, row, col] = (dst % P).astype(np.float32)
    ew_s[core_of, row, col] = ew

    taus = np.arange(NT)
    for c in range(N_CORES):
        dstg_s[c, :, :] = (c * span + taus[None, :] * P + np.arange(P)[:, None])
    dstg_s = np.minimum(dstg_s, NP - 1)

    x_pad = np.zeros((NP, IN), np.float32)
    x_pad[:N] = x
    return dict(N=N, IN=IN, E=E, NT=NT, NP=NP, span=span, T=T,
                m_tiles=m_tiles, col0=col0, x_pad=x_pad,
                src_s=src_s, dst_s=dst_s.astype(ml_dtypes.bfloat16),
                ew_s=ew_s, dstg_s=dstg_s)


def _consts(HEADS, HID):
    iota = np.tile(np.arange(P, dtype=np.float32), (P, 1)).astype(ml_dtypes.bfloat16)
    ident = np.eye(P, dtype=np.float32).astype(ml_dtypes.bfloat16)
    return iota, ident


def _build_neff1(pp, HEADS, HID):
    NT, NP, T = pp["NT"], pp["NP"], pp["T"]
    IN = pp["IN"]
    HC = HEADS * HID
    m_tiles = pp["m_tiles"]
    col0 = pp["col0"]
    M = int(m_tiles.max())
    SB = 8
    ROWB = 2 * HC  # bf16 row elems (xp | a_s,a_d f32 in tail)

    nc = bacc.Bacc("TRN2", target_bir_lowering=False, debug=False,
                   num_devices=N_CORES)
    x_h = nc.dram_tensor("x", [NP, IN], F32, kind="ExternalInput")
    wg_h = nc.dram_tensor("wg", [IN, HC], F32, kind="ExternalInput")
    asr_h = nc.dram_tensor("asr", [P, HC], F32, kind="ExternalInput")
    adr_h = nc.dram_tensor("adr", [P, HC], F32, kind="ExternalInput")
    iota_h = nc.dram_tensor("iota", [P, P], BF16, kind="ExternalInput")
    id_h = nc.dram_tensor("ident", [P, P], BF16, kind="ExternalInput")
    si_h = nc.dram_tensor("srcidx", [P, T], I32, kind="ExternalInput")
    dl_h = nc.dram_tensor("dstloc", [P, T], BF16, kind="ExternalInput")
    ew_h = nc.dram_tensor("ew", [P, T], F32, kind="ExternalInput")
    dg_h = nc.dram_tensor("dstglob", [P, NT], I32, kind="ExternalInput")

    yp_h = nc.dram_tensor("yprime", [NT * P, HC], F32, kind="ExternalOutput")
    dv_h = nc.dram_tensor("dinv", [P, NT], F32, kind="ExternalOutput")

    xp_t = nc.dram_tensor("xptab", [NP, ROWB], BF16, kind="Internal")
    ad_t = nc.dram_tensor("adtab", [NP, HEADS], F32, kind="Internal")

    with tile.TileContext(nc) as tc:
        import contextlib
        with contextlib.ExitStack() as ctx:
            cpool = ctx.enter_context(tc.tile_pool(name="const", bufs=1))
            pool = ctx.enter_context(tc.tile_pool(name="work", bufs=3))
            spool = ctx.enter_context(tc.tile_pool(name="spool", bufs=M + 2))

            wg_bf = cpool.tile([IN, HC], BF16)
            tmpw = cpool.tile([IN, HC], F32)
            nc.sync.dma_start(tmpw[:], wg_h.ap()[:])
            nc.vector.tensor_copy(wg_bf[:], tmpw[:])
            asr = cpool.tile([P, HC], F32)
            nc.sync.dma_start(asr[:], asr_h.ap()[:])
            adr = cpool.tile([P, HC], F32)
            nc.sync.dma_start(adr[:], adr_h.ap()[:])
            iota = cpool.tile([P, P], BF16)
            nc.sync.dma_start(iota[:], iota_h.ap()[:])
            ident = cpool.tile([P, P], BF16)
            nc.sync.dma_start(ident[:], id_h.ap()[:])
            dinv_all = cpool.tile([P, NT], F32)

            # ---- setup: xp table ----
            n_sb = NP // (P * SB)
            setup_ctx = contextlib.ExitStack()
            ppool = setup_ctx.enter_context(
                tc.tile_pool(name="spsum", bufs=2, space="PSUM"))
            for b in range(n_sb):
                n0 = b * P * SB
                x_sb = pool.tile([P, SB, IN], F32, tag="xsb")
                nc.sync.dma_start(
                    x_sb[:],
                    x_h.ap()[n0:n0 + P * SB, :].rearrange("(j p) c -> p j c", p=P))
                x_bf = pool.tile([P, SB, IN], BF16, tag="xbf")
                nc.vector.tensor_copy(x_bf[:], x_sb[:])
                rows = pool.tile([P, SB, ROWB], BF16, tag="rows")
                xp_bf = pool.tile([P, SB, HC], BF16, tag="xpbf")
                for j in range(SB):
                    pxt = ppool.tile([P, IN], BF16, tag="pxt", space="PSUM")
                    nc.tensor.transpose(pxt[:], x_bf[:, j, :], ident[:])
                    xt = pool.tile([P, IN], BF16, tag="xt")
                    nc.vector.tensor_copy(xt[:], pxt[:])
                    pxp = ppool.tile([P, HC], F32, tag="pxp", space="PSUM")
                    nc.tensor.matmul(pxp[:], lhsT=xt[:], rhs=wg_bf[:],
                                     start=True, stop=True)
                    nc.vector.tensor_copy(xp_bf[:, j, :], pxp[:])
                nc.vector.tensor_copy(rows[:, :, 0:HC], xp_bf[:])
                tmp = pool.tile([P, SB, HC], F32, tag="tmul")
                rows_f = rows[:].bitcast(F32)  # [P, SB, HC] f32 view
                nc.vector.tensor_tensor(
                    out=tmp[:], in0=xp_bf[:],
                    in1=asr[:].unsqueeze(1).to_broadcast([P, SB, HC]),
                    op=mybir.AluOpType.mult)
                as_f = pool.tile([P, SB, HEADS], F32, tag="asf")
                nc.vector.tensor_reduce(
                    out=as_f[:],
                    in_=tmp[:].rearrange("p j (h c) -> p j h c", h=HEADS),
                    axis=mybir.AxisListType.X, op=mybir.AluOpType.add)
                nc.vector.tensor_copy(
                    rows_f[:, :, HC // 2:HC // 2 + HEADS], as_f[:])
                ad_f = pool.tile([P, SB, HEADS], F32, tag="adf")
                nc.vector.tensor_tensor(
                    out=tmp[:], in0=xp_bf[:],
                    in1=adr[:].unsqueeze(1).to_broadcast([P, SB, HC]),
                    op=mybir.AluOpType.mult)
                nc.vector.tensor_reduce(
                    out=ad_f[:],
                    in_=tmp[:].rearrange("p j (h c) -> p j h c", h=HEADS),
                    axis=mybir.AxisListType.X, op=mybir.AluOpType.add)
                nc.vector.tensor_copy(
                    rows_f[:, :, HC // 2 + HEADS:HC // 2 + 2 * HEADS], ad_f[:])
                nc.sync.dma_start(
                    xp_t.ap()[n0:n0 + P * SB, :].rearrange("(j p) c -> p j c", p=P),
                    rows[:])
                nc.sync.dma_start(
                    ad_t.ap()[n0:n0 + P * SB, :].rearrange("(j p) c -> p j c", p=P),
                    ad_f[:])

            setup_ctx.close()

            # ---- phase 1: per dst tile ----
            ppool = ctx.enter_context(tc.tile_pool(name="p1psum", bufs=2, space="PSUM"))
            ypool = ctx.enter_context(tc.tile_pool(name="ypsum", bufs=2, space="PSUM"))
            dgt = cpool.tile([P, NT], I32)
            nc.sync.dma_start(dgt[:], dg_h.ap()[:])
            for tau in range(NT):
                m = int(m_tiles[tau])
                if m == 0:
                    continue
                t0 = int(col0[tau])
                idx_t = pool.tile([P, M], I32, tag="idx")
                nc.sync.dma_start(idx_t[:, :m], si_h.ap()[:, t0:t0 + m])
                dst_t = pool.tile([P, M], BF16, tag="dst")
                nc.sync.dma_start(dst_t[:, :m], dl_h.ap()[:, t0:t0 + m])
                ew_t = pool.tile([P, M], F32, tag="ewt")
                nc.sync.dma_start(ew_t[:, :m], ew_h.ap()[:, t0:t0 + m])

                ad_g = pool.tile([P, HEADS], F32, tag="adg")
                nc.gpsimd.indirect_dma_start(
                    out=ad_g[:], out_offset=None, in_=ad_t.ap()[:],
                    in_offset=bass.IndirectOffsetOnAxis(
                        ap=dgt[:, tau:tau + 1], axis=0))
                ad_bf = pool.tile([P, HEADS], BF16, tag="adbf")
                nc.vector.tensor_copy(ad_bf[:], ad_g[:])

                gbuf = pool.tile([P, M, ROWB], BF16, tag="gbuf")
                s_tiles = []
                padt = ppool.tile([P, M * HEADS], F32, tag="padt", space="PSUM")
                for j in range(m):
                    nc.gpsimd.indirect_dma_start(
                        out=gbuf[:, j, :], out_offset=None, in_=xp_t.ap()[:],
                        in_offset=bass.IndirectOffsetOnAxis(
                            ap=idx_t[:, j:j + 1], axis=0))
                    S = spool.tile([P, P], BF16, tag="S")
                    nc.vector.tensor_tensor(
                        out=S[:], in0=dst_t[:, j:j + 1].to_broadcast([P, P]),
                        in1=iota[:], op=mybir.AluOpType.is_equal)
                    s_tiles.append(S)
                    pst = ppool.tile([P, P], BF16, tag="pst", space="PSUM")
                    nc.tensor.transpose(pst[:], S[:], ident[:])
                    st = pool.tile([P, P], BF16, tag="st")
                    nc.vector.tensor_copy(st[:], pst[:])
                    nc.tensor.matmul(
                        padt[:, j * HEADS:(j + 1) * HEADS], lhsT=st[:],
                        rhs=ad_bf[:], start=True, stop=True)
                alpha = pool.tile([P, M * HEADS], F32, tag="alpha")
                gb_f = gbuf[:].bitcast(F32)  # [P, M, HC]
                nc.vector.tensor_tensor(
                    out=alpha[:, :m * HEADS].rearrange("p (j h) -> p j h", h=HEADS),
                    in0=gb_f[:, :m, HC // 2:HC // 2 + HEADS],
                    in1=padt[:, :m * HEADS].rearrange("p (j h) -> p j h", h=HEADS),
                    op=mybir.AluOpType.add)
                lrel = pool.tile([P, M * HEADS], F32, tag="lrel")
                nc.vector.tensor_scalar_mul(lrel[:, :m * HEADS],
                                            alpha[:, :m * HEADS], NEG_SLOPE)
                nc.vector.tensor_tensor(out=lrel[:, :m * HEADS],
                                        in0=lrel[:, :m * HEADS],
                                        in1=alpha[:, :m * HEADS],
                                        op=mybir.AluOpType.max)
                q = pool.tile([P, M * HEADS], F32, tag="q")
                nc.scalar.activation(q[:, :m * HEADS], lrel[:, :m * HEADS],
                                     mybir.ActivationFunctionType.Exp)
                RW = HC + HEADS + 1  # rhs width: msg | q | ew
                rhs = pool.tile([P, M, RW + 1], BF16, tag="rhs")
                qv = q[:, :m * HEADS].rearrange("p (j h) -> p j h", h=HEADS)
                nc.vector.tensor_tensor(
                    out=rhs[:, :m, 0:HC].rearrange("p j (h c) -> p j h c", h=HEADS),
                    in0=gbuf[:, :m, 0:HC].rearrange("p j (h c) -> p j h c", h=HEADS),
                    in1=qv.unsqueeze(3).to_broadcast([P, m, HEADS, HID]),
                    op=mybir.AluOpType.mult)
                nc.vector.tensor_copy(rhs[:, :m, HC:HC + HEADS], qv)
                nc.vector.tensor_copy(rhs[:, :m, HC + HEADS:RW],
                                      ew_t[:, :m].unsqueeze(2))
                py = ypool.tile([P, RW], F32, tag="py", space="PSUM")
                for j in range(m):
                    nc.tensor.matmul(py[:], lhsT=s_tiles[j][:], rhs=rhs[:, j, :RW],
                                     start=(j == 0), stop=(j == m - 1))
                deg_s = pool.tile([P, 1], F32, tag="degs")
                nc.scalar.activation(deg_s[:], py[:, HC + HEADS:RW],
                                     mybir.ActivationFunctionType.Sqrt)
                nc.vector.reciprocal(dinv_all[:, tau:tau + 1], deg_s[:])
                rec = pool.tile([P, HEADS], F32, tag="rec")
                nc.vector.reciprocal(rec[:], py[:, HC:HC + HEADS])
                scale = pool.tile([P, HEADS], F32, tag="scale")
                nc.vector.tensor_tensor(
                    out=scale[:], in0=rec[:],
                    in1=dinv_all[:, tau:tau + 1].to_broadcast([P, HEADS]),
                    op=mybir.AluOpType.mult)
                yp = pool.tile([P, HC], F32, tag="yp")
                nc.vector.tensor_tensor(
                    out=yp[:].rearrange("p (h c) -> p h c", h=HEADS),
                    in0=py[:, 0:HC].rearrange("p (h c) -> p h c", h=HEADS),
                    in1=scale[:].unsqueeze(2).to_broadcast([P, HEADS, HID]),
                    op=mybir.AluOpType.mult)
                nc.sync.dma_start(yp_h.ap()[tau * P:(tau + 1) * P, :], yp[:])
            nc.sync.dma_start(dv_h.ap()[:], dinv_all[:])

    nc.compile()
    return nc


def _build_neff2(pp, HEADS, HID, OUT):
    NT, NP, T = pp["NT"], pp["NP"], pp["T"]
    HC = HEADS * HID
    m_tiles = pp["m_tiles"]
    col0 = pp["col0"]
    M = int(m_tiles.max())

    nc = bacc.Bacc("TRN2", target_bir_lowering=False, debug=False,
                   num_devices=N_CORES)
    yf_h = nc.dram_tensor("yfull", [NP, HC], F32, kind="ExternalInput")
    wc_h = nc.dram_tensor("wgcn", [HC, OUT], F32, kind="ExternalInput")
    iota_h = nc.dram_tensor("iota", [P, P], BF16, kind="ExternalInput")
    id_h = nc.dram_tensor("ident", [P, P], BF16, kind="ExternalInput")
    si_h = nc.dram_tensor("srcidx", [P, T], I32, kind="ExternalInput")
    dl_h = nc.dram_tensor("dstloc", [P, T], BF16, kind="ExternalInput")
    ew_h = nc.dram_tensor("ew", [P, T], F32, kind="ExternalInput")
    dv_h = nc.dram_tensor("dinv", [P, NT], F32, kind="ExternalInput")
    out_h = nc.dram_tensor("outp", [NT * P, OUT], F32, kind="ExternalOutput")

    with tile.TileContext(nc) as tc:
        import contextlib
        with contextlib.ExitStack() as ctx:
            cpool = ctx.enter_context(tc.tile_pool(name="const", bufs=1))
            pool = ctx.enter_context(tc.tile_pool(name="work", bufs=3))
            spool = ctx.enter_context(tc.tile_pool(name="spool", bufs=M + 2))
            ppool = ctx.enter_context(tc.tile_pool(name="psum", bufs=2, space="PSUM"))

            wc_bf = cpool.tile([HC, OUT], BF16)
            tmpw = cpool.tile([HC, OUT], F32)
            nc.sync.dma_start(tmpw[:], wc_h.ap()[:])
            nc.vector.tensor_copy(wc_bf[:], tmpw[:])
            iota = cpool.tile([P, P], BF16)
            nc.sync.dma_start(iota[:], iota_h.ap()[:])
            ident = cpool.tile([P, P], BF16)
            nc.sync.dma_start(ident[:], id_h.ap()[:])
            dinv = cpool.tile([P, NT], F32)
            nc.sync.dma_start(dinv[:], dv_h.ap()[:])

            for tau in range(NT):
                m = int(m_tiles[tau])
                if m == 0:
                    continue
                t0 = int(col0[tau])
                idx_t = pool.tile([P, M], I32, tag="idx")
                nc.sync.dma_start(idx_t[:, :m], si_h.ap()[:, t0:t0 + m])
                dst_t = pool.tile([P, M], BF16, tag="dst")
                nc.sync.dma_start(dst_t[:, :m], dl_h.ap()[:, t0:t0 + m])
                ew_t = pool.tile([P, M], F32, tag="ewt")
                nc.sync.dma_start(ew_t[:, :m], ew_h.ap()[:, t0:t0 + m])

                g2 = pool.tile([P, M, HC], F32, tag="g2")
                s_tiles = []
                for j in range(m):
                    nc.gpsimd.indirect_dma_start(
                        out=g2[:, j, :], out_offset=None, in_=yf_h.ap()[:],
                        in_offset=bass.IndirectOffsetOnAxis(
                            ap=idx_t[:, j:j + 1], axis=0))
                    S = spool.tile([P, P], BF16, tag="S")
                    nc.vector.tensor_tensor(
                        out=S[:], in0=dst_t[:, j:j + 1].to_broadcast([P, P]),
                        in1=iota[:], op=mybir.AluOpType.is_equal)
                    s_tiles.append(S)
                rhs = pool.tile([P, M, HC], BF16, tag="rhs")
                nc.vector.tensor_tensor(
                    out=rhs[:, :m, :], in0=g2[:, :m, :],
                    in1=ew_t[:, :m].unsqueeze(2).to_broadcast([P, m, HC]),
                    op=mybir.AluOpType.mult)
                pz = ppool.tile([P, HC], F32, tag="pz", space="PSUM")
                for j in range(m):
                    nc.tensor.matmul(pz[:], lhsT=s_tiles[j][:], rhs=rhs[:, j, :],
                                     start=(j == 0), stop=(j == m - 1))
                zb = pool.tile([P, HC], BF16, tag="zb")
                nc.vector.tensor_tensor(
                    out=zb[:], in0=pz[:],
                    in1=dinv[:, tau:tau + 1].to_broadcast([P, HC]),
                    op=mybir.AluOpType.mult)
                pzt = ppool.tile([P, HC], BF16, tag="pzt", space="PSUM")
                nc.tensor.transpose(pzt[:], zb[:], ident[:])
                zt = pool.tile([P, HC], BF16, tag="zt")
                nc.vector.tensor_copy(zt[:], pzt[:])
                po = ppool.tile([P, OUT], F32, tag="po", space="PSUM")
                nc.tensor.matmul(po[:], lhsT=zt[:], rhs=wc_bf[:],
                                 start=True, stop=True)
                osb = pool.tile([P, OUT], F32, tag="osb")
                nc.vector.tensor_copy(osb[:], po[:])
                nc.sync.dma_start(out_h.ap()[tau * P:(tau + 1) * P, :], osb[:])

    nc.compile()
    return nc


def kernel(x, edge_index, edge_weight, W_gat, att_src, att_dst, W_gcn):
    x = np.asarray(x, np.float32)
    edge_index = np.asarray(edge_index)
    edge_weight = np.asarray(edge_weight, np.float32)
    W_gat = np.asarray(W_gat, np.float32)
    att_src = np.asarray(att_src, np.float32)
    att_dst = np.asarray(att_dst, np.float32)
    W_gcn = np.asarray(W_gcn, np.float32)

    HEADS, HID = att_src.shape
    HC = HEADS * HID
    OUT = W_gcn.shape[1]
    N = x.shape[0]

    pp = _prep(x, edge_index, edge_weight)
    iota, ident = _consts(HEADS, HID)
    asr = np.tile(att_src.reshape(1, HC), (P, 1)).astype(np.float32)
    adr = np.tile(att_dst.reshape(1, HC), (P, 1)).astype(np.float32)

    nc1 = _build_neff1(pp, HEADS, HID)
    in1 = []
    for c in range(N_CORES):
        in1.append({
            "x": pp["x_pad"], "wg": W_gat, "asr": asr, "adr": adr,
            "iota": iota, "ident": ident,
            "srcidx": pp["src_s"][c], "dstloc": pp["dst_s"][c],
            "ew": pp["ew_s"][c], "dstglob": pp["dstg_s"][c],
        })
    res1 = run_bass_kernel_spmd(nc1, in1, core_ids=list(range(N_CORES)),
                                trace=True)
    LAST_EXEC_NS[0] = res1.exec_time_ns

    yfull = np.concatenate([res1.results[c]["yprime"] for c in range(N_CORES)],
                           axis=0).astype(np.float32)
    DEBUG["yfull"] = yfull
    DEBUG["dinv"] = [res1.results[c]["dinv"] for c in range(N_CORES)]
    DEBUG["pp"] = pp

    nc2 = _build_neff2(pp, HEADS, HID, OUT)
    in2 = []
    for c in range(N_CORES):
        in2.append({
            "yfull": yfull, "wgcn": W_gcn, "iota": iota, "ident": ident,
            "srcidx": pp["src_s"][c], "dstloc": pp["dst_s"][c],
            "ew": pp["ew_s"][c], "dinv": res1.results[c]["dinv"],
        })
    res2 = run_bass_kernel_spmd(nc2, in2, core_ids=list(range(N_CORES)),
                                trace=True)
    LAST_EXEC_NS[1] = res2.exec_time_ns

    out = np.concatenate([res2.results[c]["outp"] for c in range(N_CORES)],
                         axis=0)[:N]
    return out.astype(np.float32)


# revision 7
# speedup vs baseline: 1.0708x; 1.0708x over previous
"""GAT(4 heads, concat) -> GCN fused kernel for trn2 x 8 NeuronCores, v2.

v2: per-edge source rows fetched with batched SWDGE dma_gather (custom
gpsimd descriptor generation, thousands of rows per instruction) from a
replicated node table split in 4 chunks (int16 index range). Segment
sums are one-hot scatter matmuls accumulating in PSUM.
"""
import sys, types, math

sys.path.insert(0, "/opt/trn_rl_repo")
import numpy as np
import ml_dtypes

if "antenv.axon_hooks" not in sys.modules:
    _m = types.ModuleType("antenv.axon_hooks")
    _box = [None]
    _m.set_axon_ntff_profile_hook = lambda h: _box.__setitem__(0, h)
    _m.get_axon_ntff_profile_hook = lambda: _box[0]
    sys.modules["antenv.axon_hooks"] = _m
    try:
        from trn_agent_boot.trn_boot import _ntff_profile_via_ctypes
        _m.set_axon_ntff_profile_hook(
            _ntff_profile_via_ctypes("/opt/axon/libaxon_pjrt.so"))
    except Exception:
        pass

import concourse.bass as bass
import concourse.tile as tile
from concourse import bacc, mybir
from concourse.bass_utils import run_bass_kernel_spmd

F32 = mybir.dt.float32
BF16 = mybir.dt.bfloat16
I32 = mybir.dt.int32
I16 = mybir.dt.int16

N_CORES = 8
P = 128
NCHUNK = 4     # src-index chunks (int16 gather index range)
GS = 4         # dst tiles per gather group
NEG_SLOPE = 0.2
LAST_EXEC_NS = [None, None]
DEBUG = {}


def _prep(x, edge_index, edge_weight):
    N, IN = x.shape
    src = np.concatenate([edge_index[0], np.arange(N, dtype=np.int64)])
    dst = np.concatenate([edge_index[1], np.arange(N, dtype=np.int64)])
    ew = np.concatenate([edge_weight.astype(np.float32), np.ones(N, np.float32)])

    NT = math.ceil(N / (P * N_CORES))
    NP = NT * P * N_CORES
    span = NT * P
    CH = max(1, NP // NCHUNK)
    assert CH < 2 ** 15

    core_of = dst // span
    tau_of = (dst % span) // P
    chunk_of = src // CH
    order = np.lexsort((src, tau_of, core_of))
    src, dst, ew = src[order], dst[order], ew[order]
    core_of, tau_of, chunk_of = core_of[order], tau_of[order], chunk_of[order]

    counts = np.zeros((N_CORES, NT, NCHUNK), np.int64)
    np.add.at(counts, (core_of, tau_of, chunk_of), 1)
    m4 = np.ceil(counts.max(axis=0) / P).astype(np.int64)   # [NT, NCHUNK]

    n_groups = math.ceil(NT / GS)
    CB = np.zeros((NT, NCHUNK), np.int64)
    seg0 = np.zeros((n_groups, NCHUNK), np.int64)
    segn = np.zeros((n_groups, NCHUNK), np.int64)
    pos = 0
    for g in range(n_groups):
        taus = range(g * GS, min((g + 1) * GS, NT))
        for c in range(NCHUNK):
            seg0[g, c] = pos
            for t in taus:
                CB[t, c] = pos
                pos += int(m4[t, c])
            segn[g, c] = pos - seg0[g, c]
    T = int(pos)

    src_s = np.zeros((N_CORES, P, T), np.int16)
    dst_s = np.full((N_CORES, P, T), 200.0, np.float32)
    ew_s = np.zeros((N_CORES, P, T), np.float32)
    dstg_s = np.zeros((N_CORES, P, NT), np.int32)

    flat = (core_of * NT + tau_of) * NCHUNK + chunk_of
    first = np.concatenate(
        [[0], np.cumsum(np.bincount(flat, minlength=N_CORES * NT * NCHUNK))])[:-1]
    rank = np.arange(len(src)) - first[flat]
    col = CB[tau_of, chunk_of] + rank // P
    row = rank % P
    src_s[core_of, row, col] = (src - chunk_of * CH).astype(np.int16)
    dst_s[core_of, row, col] = (dst % P).astype(np.float32)
    ew_s[core_of, row, col] = ew

    # idx16: dma_gather wrapped layout. Segment = (group, chunk); position
    # j within segment at [j%16, 8*seg_base + j//16]; j = (t-seg_base)*128+p.
    segbase_of_col = np.zeros(T, np.int64)
    for g in range(n_groups):
        for c in range(NCHUNK):
            s0, sn = int(seg0[g, c]), int(segn[g, c])
            segbase_of_col[s0:s0 + sn] = s0
    tcols = np.arange(T)
    jpos = (tcols[None, :] - segbase_of_col[None, :]) * P + np.arange(P)[:, None]
    icol = (8 * segbase_of_col[None, :] + jpos // 16).astype(np.int64)
    irow = (jpos % 16).astype(np.int64)
    idx16 = np.zeros((N_CORES, 16, 8 * T), np.int16)
    for c_ in range(N_CORES):
        idx16[c_, irow.ravel(), icol.ravel()] = src_s[c_].ravel()
    idx16 = np.tile(idx16, (1, 8, 1))

    taus_ar = np.arange(NT)
    for c_ in range(N_CORES):
        dstg_s[c_, :, :] = (c_ * span + taus_ar[None, :] * P
                            + np.arange(P)[:, None])
    dstg_s = np.minimum(dstg_s, NP - 1)

    x_pad = np.zeros((NP, IN), np.float32)
    x_pad[:N] = x
    return dict(N=N, IN=IN, NT=NT, NP=NP, span=span, T=T, CH=CH,
                m4=m4, CB=CB, seg0=seg0, segn=segn, n_groups=n_groups,
                x_pad=x_pad, src_s=src_s, idx16=idx16,
                dst_s=dst_s.astype(ml_dtypes.bfloat16), ew_s=ew_s,
                dstg_s=dstg_s)


def _consts():
    iota = np.tile(np.arange(P, dtype=np.float32), (P, 1)).astype(ml_dtypes.bfloat16)
    ident = np.eye(P, dtype=np.float32).astype(ml_dtypes.bfloat16)
    return iota, ident


def _phase_tiles(pp):
    NT, m4, CB = pp["NT"], pp["m4"], pp["CB"]
    cols = []
    for t in range(NT):
        lst = []
        for c in range(NCHUNK):
            lst += [int(CB[t, c]) + k for k in range(int(m4[t, c]))]
        cols.append(lst)
    return cols


def _runs(cols):
    """Contiguous (col0, n, ji0) runs of a sorted tile-column list."""
    runs = []
    i = 0
    while i < len(cols):
        j = i
        while j + 1 < len(cols) and cols[j + 1] == cols[j] + 1:
            j += 1
        runs.append((cols[i], j - i + 1, i))
        i = j + 1
    return runs


def _build_neff1(pp, HEADS, HID):
    NT, NP, T, CH = pp["NT"], pp["NP"], pp["T"], pp["CH"]
    IN = pp["IN"]
    HC = HEADS * HID
    seg0, segn, n_groups = pp["seg0"], pp["segn"], pp["n_groups"]
    tilecols = _phase_tiles(pp)
    SB = 8
    ROWB = 2 * HC
    TGmax = max(int(segn[g].sum()) for g in range(n_groups))
    MM = max(len(c) for c in tilecols)

    nc = bacc.Bacc("TRN2", target_bir_lowering=False, debug=False,
                   num_devices=N_CORES, num_swdge_queues=4)
    x_h = nc.dram_tensor("x", [NP, IN], F32, kind="ExternalInput")
    wg_h = nc.dram_tensor("wg", [IN, HC], F32, kind="ExternalInput")
    asr_h = nc.dram_tensor("asr", [P, HC], F32, kind="ExternalInput")
    adr_h = nc.dram_tensor("adr", [P, HC], F32, kind="ExternalInput")
    iota_h = nc.dram_tensor("iota", [P, P], BF16, kind="ExternalInput")
    id_h = nc.dram_tensor("ident", [P, P], BF16, kind="ExternalInput")
    ix_h = nc.dram_tensor("idx16", [P, 8 * T], I16, kind="ExternalInput")
    dl_h = nc.dram_tensor("dstloc", [P, T], BF16, kind="ExternalInput")
    ew_h = nc.dram_tensor("ew", [P, T], F32, kind="ExternalInput")
    dg_h = nc.dram_tensor("dstglob", [P, NT], I32, kind="ExternalInput")

    yp_h = nc.dram_tensor("yprime", [NT * P, HC], F32, kind="ExternalOutput")
    dv_h = nc.dram_tensor("dinv", [P, NT], F32, kind="ExternalOutput")

    xp_t = nc.dram_tensor("xptab", [NP, ROWB], BF16, kind="Internal")
    ad_t = nc.dram_tensor("adtab", [NP, HEADS], F32, kind="Internal")

    with tile.TileContext(nc) as tc:
        import contextlib
        with contextlib.ExitStack() as ctx:
            cpool = ctx.enter_context(tc.tile_pool(name="const", bufs=1))
            pool = ctx.enter_context(tc.tile_pool(name="work", bufs=3))
            gpool = ctx.enter_context(tc.tile_pool(name="gath", bufs=2))
            spool = ctx.enter_context(tc.tile_pool(name="spool", bufs=2))

            wg_bf = cpool.tile([IN, HC], BF16)
            tmpw = cpool.tile([IN, HC], F32)
            nc.sync.dma_start(tmpw[:], wg_h.ap()[:])
            nc.vector.tensor_copy(wg_bf[:], tmpw[:])
            asr = cpool.tile([P, HC], F32)
            nc.sync.dma_start(asr[:], asr_h.ap()[:])
            adr = cpool.tile([P, HC], F32)
            nc.sync.dma_start(adr[:], adr_h.ap()[:])
            iota = cpool.tile([P, P], BF16)
            nc.sync.dma_start(iota[:], iota_h.ap()[:])
            ident = cpool.tile([P, P], BF16)
            nc.sync.dma_start(ident[:], id_h.ap()[:])
            dinv_all = cpool.tile([P, NT], F32)

            # ---- setup: xp/a_s/a_d tables ----
            n_sb = NP // (P * SB)
            setup_ctx = contextlib.ExitStack()
            spool_s = setup_ctx.enter_context(tc.tile_pool(name="swork", bufs=3))
            ppool = setup_ctx.enter_context(
                tc.tile_pool(name="spsum", bufs=2, space="PSUM"))
            for b in range(n_sb):
                n0 = b * P * SB
                x_sb = spool_s.tile([P, SB, IN], F32, tag="xsb")
                nc.sync.dma_start(
                    x_sb[:],
                    x_h.ap()[n0:n0 + P * SB, :].rearrange("(j p) c -> p j c", p=P))
                x_bf = spool_s.tile([P, SB, IN], BF16, tag="xbf")
                nc.vector.tensor_copy(x_bf[:], x_sb[:])
                rows = spool_s.tile([P, SB, ROWB], BF16, tag="rows")
                xp_bf = spool_s.tile([P, SB, HC], BF16, tag="xpbf")
                for j in range(SB):
                    pxt = ppool.tile([P, IN], BF16, tag="pxt", space="PSUM")
                    nc.tensor.transpose(pxt[:], x_bf[:, j, :], ident[:])
                    xt = spool_s.tile([P, IN], BF16, tag="xt")
                    nc.vector.tensor_copy(xt[:], pxt[:])
                    pxp = ppool.tile([P, HC], F32, tag="pxp", space="PSUM")
                    nc.tensor.matmul(pxp[:], lhsT=xt[:], rhs=wg_bf[:],
                                     start=True, stop=True)
                    nc.vector.tensor_copy(xp_bf[:, j, :], pxp[:])
                nc.vector.tensor_copy(rows[:, :, 0:HC], xp_bf[:])
                tmp = spool_s.tile([P, SB, HC], F32, tag="tmul")
                rows_f = rows[:].bitcast(F32)
                nc.vector.tensor_tensor(
                    out=tmp[:], in0=xp_bf[:],
                    in1=asr[:].unsqueeze(1).to_broadcast([P, SB, HC]),
                    op=mybir.AluOpType.mult)
                as_f = spool_s.tile([P, SB, HEADS], F32, tag="asf")
                nc.vector.tensor_reduce(
                    out=as_f[:],
                    in_=tmp[:].rearrange("p j (h c) -> p j h c", h=HEADS),
                    axis=mybir.AxisListType.X, op=mybir.AluOpType.add)
                nc.vector.tensor_copy(
                    rows_f[:, :, HC // 2:HC // 2 + HEADS], as_f[:])
                ad_f = spool_s.tile([P, SB, HEADS], F32, tag="adf")
                nc.vector.tensor_tensor(
                    out=tmp[:], in0=xp_bf[:],
                    in1=adr[:].unsqueeze(1).to_broadcast([P, SB, HC]),
                    op=mybir.AluOpType.mult)
                nc.vector.tensor_reduce(
                    out=ad_f[:],
                    in_=tmp[:].rearrange("p j (h c) -> p j h c", h=HEADS),
                    axis=mybir.AxisListType.X, op=mybir.AluOpType.add)
                nc.vector.tensor_copy(
                    rows_f[:, :, HC // 2 + HEADS:HC // 2 + 2 * HEADS], ad_f[:])
                nc.sync.dma_start(
                    xp_t.ap()[n0:n0 + P * SB, :].rearrange("(j p) c -> p j c", p=P),
                    rows[:])
                nc.sync.dma_start(
                    ad_t.ap()[n0:n0 + P * SB, :].rearrange("(j p) c -> p j c", p=P),
                    ad_f[:])
            setup_ctx.close()

            # ---- phase 1 ----
            ppool = ctx.enter_context(
                tc.tile_pool(name="p1psum", bufs=2, space="PSUM"))
            ypool = ctx.enter_context(
                tc.tile_pool(name="ypsum", bufs=2, space="PSUM"))
            dgt = cpool.tile([P, NT], I32)
            nc.sync.dma_start(dgt[:], dg_h.ap()[:])
            RW = HC + HEADS + 1

            for g in range(n_groups):
                taus = list(range(g * GS, min((g + 1) * GS, NT)))
                g0 = int(seg0[g, 0])
                gn = int(segn[g].sum())
                if gn == 0:
                    continue
                gbuf = gpool.tile([P, TGmax, ROWB], BF16, tag="gbuf")
                qrr = g % 4
                for c in range(NCHUNK):
                    s0, sn = int(seg0[g, c]), int(segn[g, c])
                    if sn == 0:
                        continue
                    ixt = pool.tile([P, 8 * TGmax], I16, tag="ixt")
                    nc.sync.dma_start(ixt[:, :8 * sn],
                                      ix_h.ap()[:, 8 * s0:8 * (s0 + sn)])
                    for k0 in range(0, sn, 8):
                        kn = min(8, sn - k0)
                        nidx = kn * P
                        nc.gpsimd.dma_gather(
                            out_ap=gbuf[:, s0 - g0 + k0:s0 - g0 + k0 + kn, :],
                            in_ap=xp_t.ap()[c * CH:, :],
                            idxs_ap=ixt[:, 8 * k0:8 * (k0 + kn)],
                            num_idxs=nidx, num_idxs_reg=nidx, elem_size=ROWB,
                            queue_num=qrr)
                        qrr = (qrr + 1) % 4
                dst_t = pool.tile([P, TGmax], BF16, tag="dst")
                nc.sync.dma_start(dst_t[:, :gn], dl_h.ap()[:, g0:g0 + gn])
                ew_t = pool.tile([P, TGmax], F32, tag="ewt")
                nc.sync.dma_start(ew_t[:, :gn], ew_h.ap()[:, g0:g0 + gn])

                for tau in taus:
                    cols = tilecols[tau]
                    m = len(cols)
                    if m == 0:
                        continue
                    runs = _runs(cols)
                    ad_g = pool.tile([P, HEADS], F32, tag="adg")
                    nc.gpsimd.indirect_dma_start(
                        out=ad_g[:], out_offset=None, in_=ad_t.ap()[:],
                        in_offset=bass.IndirectOffsetOnAxis(
                            ap=dgt[:, tau:tau + 1], axis=0))
                    ad_bf = pool.tile([P, HEADS], BF16, tag="adbf")
                    nc.vector.tensor_copy(ad_bf[:], ad_g[:])

                    S_all = spool.tile([P, MM, P], BF16, tag="S")
                    for (t0, nrun, ji0) in runs:
                        nc.vector.tensor_tensor(
                            out=S_all[:, ji0:ji0 + nrun, :],
                            in0=dst_t[:, t0 - g0:t0 - g0 + nrun]
                                .unsqueeze(2).to_broadcast([P, nrun, P]),
                            in1=iota[:].unsqueeze(1).to_broadcast([P, nrun, P]),
                            op=mybir.AluOpType.is_equal)
                    st_all = pool.tile([P, MM, P], BF16, tag="st")
                    padt = ppool.tile([P, MM * HEADS], F32, tag="padt",
                                      space="PSUM")
                    for b0 in range(0, m, 8):
                        bn = min(8, m - b0)
                        pst = ppool.tile([P, 8, P], BF16, tag="pst",
                                         space="PSUM")
                        for j in range(bn):
                            nc.tensor.transpose(pst[:, j, :],
                                                S_all[:, b0 + j, :], ident[:])
                        nc.vector.tensor_copy(st_all[:, b0:b0 + bn, :],
                                              pst[:, :bn, :])
                    for ji in range(m):
                        nc.tensor.matmul(
                            padt[:, ji * HEADS:(ji + 1) * HEADS],
                            lhsT=st_all[:, ji, :],
                            rhs=ad_bf[:], start=True, stop=True)
                    alpha = pool.tile([P, MM * HEADS], F32, tag="alpha")
                    gb_f = gbuf[:].bitcast(F32)
                    for (t0, nrun, ji0) in runs:
                        nc.vector.tensor_tensor(
                            out=alpha[:, ji0 * HEADS:(ji0 + nrun) * HEADS]
                                .rearrange("p (j h) -> p j h", h=HEADS),
                            in0=gb_f[:, t0 - g0:t0 - g0 + nrun,
                                     HC // 2:HC // 2 + HEADS],
                            in1=padt[:, ji0 * HEADS:(ji0 + nrun) * HEADS]
                                .rearrange("p (j h) -> p j h", h=HEADS),
                            op=mybir.AluOpType.add)
                    lrel = pool.tile([P, MM * HEADS], F32, tag="lrel")
                    nc.vector.tensor_scalar_mul(lrel[:, :m * HEADS],
                                                alpha[:, :m * HEADS], NEG_SLOPE)
                    nc.vector.tensor_tensor(out=lrel[:, :m * HEADS],
                                            in0=lrel[:, :m * HEADS],
                                            in1=alpha[:, :m * HEADS],
                                            op=mybir.AluOpType.max)
                    q = pool.tile([P, MM * HEADS], F32, tag="q")
                    nc.scalar.activation(q[:, :m * HEADS], lrel[:, :m * HEADS],
                                         mybir.ActivationFunctionType.Exp)
                    rhs = pool.tile([P, MM, RW + 1], BF16, tag="rhs")
                    qv = q[:, :m * HEADS].rearrange("p (j h) -> p j h", h=HEADS)
                    for (t0, nrun, ji0) in runs:
                        nc.vector.tensor_tensor(
                            out=rhs[:, ji0:ji0 + nrun, 0:HC]
                                .rearrange("p j (h c) -> p j h c", h=HEADS),
                            in0=gbuf[:, t0 - g0:t0 - g0 + nrun, 0:HC]
                                .rearrange("p j (h c) -> p j h c", h=HEADS),
                            in1=q[:, ji0 * HEADS:(ji0 + nrun) * HEADS]
                                .rearrange("p (j h) -> p j h", h=HEADS)
                                .unsqueeze(3).to_broadcast([P, nrun, HEADS, HID]),
                            op=mybir.AluOpType.mult)
                        nc.vector.tensor_copy(
                            rhs[:, ji0:ji0 + nrun, HC + HEADS:RW],
                            ew_t[:, t0 - g0:t0 - g0 + nrun].unsqueeze(2))
                    nc.vector.tensor_copy(rhs[:, :m, HC:HC + HEADS], qv)
                    py = ypool.tile([P, RW], F32, tag="py", space="PSUM")
                    for ji in range(m):
                        nc.tensor.matmul(py[:], lhsT=S_all[:, ji, :],
                                         rhs=rhs[:, ji, :RW],
                                         start=(ji == 0), stop=(ji == m - 1))
                    deg_s = pool.tile([P, 1], F32, tag="degs")
                    nc.scalar.activation(deg_s[:], py[:, HC + HEADS:RW],
                                         mybir.ActivationFunctionType.Sqrt)
                    nc.vector.reciprocal(dinv_all[:, tau:tau + 1], deg_s[:])
                    rec = pool.tile([P, HEADS], F32, tag="rec")
                    nc.vector.reciprocal(rec[:], py[:, HC:HC + HEADS])
                    scale = pool.tile([P, HEADS], F32, tag="scale")
                    nc.vector.tensor_tensor(
                        out=scale[:], in0=rec[:],
                        in1=dinv_all[:, tau:tau + 1].to_broadcast([P, HEADS]),
                        op=mybir.AluOpType.mult)
                    yp = pool.tile([P, HC], F32, tag="yp")
                    nc.vector.tensor_tensor(
                        out=yp[:].rearrange("p (h c) -> p h c", h=HEADS),
                        in0=py[:, 0:HC].rearrange("p (h c) -> p h c", h=HEADS),
                        in1=scale[:].unsqueeze(2).to_broadcast([P, HEADS, HID]),
                        op=mybir.AluOpType.mult)
                    nc.sync.dma_start(yp_h.ap()[tau * P:(tau + 1) * P, :], yp[:])
            nc.sync.dma_start(dv_h.ap()[:], dinv_all[:])

    nc.compile()
    return nc


def _build_neff2(pp, HEADS, HID, OUT):
    NT, NP, T, CH = pp["NT"], pp["NP"], pp["T"], pp["CH"]
    HC = HEADS * HID
    seg0, segn, n_groups = pp["seg0"], pp["segn"], pp["n_groups"]
    tilecols = _phase_tiles(pp)
    TGmax = max(int(segn[g].sum()) for g in range(n_groups))
    MM = max(len(c) for c in tilecols)

    nc = bacc.Bacc("TRN2", target_bir_lowering=False, debug=False,
                   num_devices=N_CORES, num_swdge_queues=4)
    yf_h = nc.dram_tensor("yfull", [NP, HC], F32, kind="ExternalInput")
    wc_h = nc.dram_tensor("wgcn", [HC, OUT], F32, kind="ExternalInput")
    iota_h = nc.dram_tensor("iota", [P, P], BF16, kind="ExternalInput")
    id_h = nc.dram_tensor("ident", [P, P], BF16, kind="ExternalInput")
    ix_h = nc.dram_tensor("idx16", [P, 8 * T], I16, kind="ExternalInput")
    dl_h = nc.dram_tensor("dstloc", [P, T], BF16, kind="ExternalInput")
    ew_h = nc.dram_tensor("ew", [P, T], F32, kind="ExternalInput")
    dv_h = nc.dram_tensor("dinv", [P, NT], F32, kind="ExternalInput")
    out_h = nc.dram_tensor("outp", [NT * P, OUT], F32, kind="ExternalOutput")

    with tile.TileContext(nc) as tc:
        import contextlib
        with contextlib.ExitStack() as ctx:
            cpool = ctx.enter_context(tc.tile_pool(name="const", bufs=1))
            pool = ctx.enter_context(tc.tile_pool(name="work", bufs=3))
            gpool = ctx.enter_context(tc.tile_pool(name="gath", bufs=2))
            spool = ctx.enter_context(tc.tile_pool(name="spool", bufs=2))
            ppool = ctx.enter_context(
                tc.tile_pool(name="psum", bufs=2, space="PSUM"))

            wc_bf = cpool.tile([HC, OUT], BF16)
            tmpw = cpool.tile([HC, OUT], F32)
            nc.sync.dma_start(tmpw[:], wc_h.ap()[:])
            nc.vector.tensor_copy(wc_bf[:], tmpw[:])
            iota = cpool.tile([P, P], BF16)
            nc.sync.dma_start(iota[:], iota_h.ap()[:])
            ident = cpool.tile([P, P], BF16)
            nc.sync.dma_start(ident[:], id_h.ap()[:])
            dinv = cpool.tile([P, NT], F32)
            nc.sync.dma_start(dinv[:], dv_h.ap()[:])

            for g in range(n_groups):
                taus = list(range(g * GS, min((g + 1) * GS, NT)))
                g0 = int(seg0[g, 0])
                gn = int(segn[g].sum())
                if gn == 0:
                    continue
                gbuf = gpool.tile([P, TGmax, HC], F32, tag="gbuf")
                qrr = g % 4
                for c in range(NCHUNK):
                    s0, sn = int(seg0[g, c]), int(segn[g, c])
                    if sn == 0:
                        continue
                    ixt = pool.tile([P, 8 * TGmax], I16, tag="ixt")
                    nc.sync.dma_start(ixt[:, :8 * sn],
                                      ix_h.ap()[:, 8 * s0:8 * (s0 + sn)])
                    for k0 in range(0, sn, 8):
                        kn = min(8, sn - k0)
                        nidx = kn * P
                        nc.gpsimd.dma_gather(
                            out_ap=gbuf[:, s0 - g0 + k0:s0 - g0 + k0 + kn, :],
                            in_ap=yf_h.ap()[c * CH:, :],
                            idxs_ap=ixt[:, 8 * k0:8 * (k0 + kn)],
                            num_idxs=nidx, num_idxs_reg=nidx, elem_size=HC,
                            queue_num=qrr)
                        qrr = (qrr + 1) % 4
                dst_t = pool.tile([P, TGmax], BF16, tag="dst")
                nc.sync.dma_start(dst_t[:, :gn], dl_h.ap()[:, g0:g0 + gn])
                ew_t = pool.tile([P, TGmax], F32, tag="ewt")
                nc.sync.dma_start(ew_t[:, :gn], ew_h.ap()[:, g0:g0 + gn])

                rhs = gpool.tile([P, TGmax, HC], BF16, tag="rhs")
                nc.vector.tensor_tensor(
                    out=rhs[:, :gn, :], in0=gbuf[:, :gn, :],
                    in1=ew_t[:, :gn].unsqueeze(2).to_broadcast([P, gn, HC]),
                    op=mybir.AluOpType.mult)

                for tau in taus:
                    cols = tilecols[tau]
                    m = len(cols)
                    if m == 0:
                        continue
                    runs = _runs(cols)
                    S_all = spool.tile([P, MM, P], BF16, tag="S")
                    for (t0, nrun, ji0) in runs:
                        nc.vector.tensor_tensor(
                            out=S_all[:, ji0:ji0 + nrun, :],
                            in0=dst_t[:, t0 - g0:t0 - g0 + nrun]
                                .unsqueeze(2).to_broadcast([P, nrun, P]),
                            in1=iota[:].unsqueeze(1).to_broadcast([P, nrun, P]),
                            op=mybir.AluOpType.is_equal)
                    pz = ppool.tile([P, HC], F32, tag="pz", space="PSUM")
                    for ji, t in enumerate(cols):
                        lt = t - g0
                        nc.tensor.matmul(pz[:], lhsT=S_all[:, ji, :],
                                         rhs=rhs[:, lt, :],
                                         start=(ji == 0), stop=(ji == m - 1))
                    zb = pool.tile([P, HC], BF16, tag="zb")
                    nc.vector.tensor_tensor(
                        out=zb[:], in0=pz[:],
                        in1=dinv[:, tau:tau + 1].to_broadcast([P, HC]),
                        op=mybir.AluOpType.mult)
                    pzt = ppool.tile([P, HC], BF16, tag="pzt", space="PSUM")
                    nc.tensor.transpose(pzt[:], zb[:], ident[:])
                    zt = pool.tile([P, HC], BF16, tag="zt")
                    nc.vector.tensor_copy(zt[:], pzt[:])
                    po = ppool.tile([P, OUT], F32, tag="po", space="PSUM")
                    nc.tensor.matmul(po[:], lhsT=zt[:], rhs=wc_bf[:],
                                     start=True, stop=True)
                    osb = pool.tile([P, OUT], F32, tag="osb")
                    nc.vector.tensor_copy(osb[:], po[:])
                    nc.sync.dma_start(out_h.ap()[tau * P:(tau + 1) * P, :],
                                      osb[:])

    nc.compile()
    return nc


def kernel(x, edge_index, edge_weight, W_gat, att_src, att_dst, W_gcn):
    x = np.asarray(x, np.float32)
    edge_index = np.asarray(edge_index)
    edge_weight = np.asarray(edge_weight, np.float32)
    W_gat = np.asarray(W_gat, np.float32)
    att_src = np.asarray(att_src, np.float32)
    att_dst = np.asarray(att_dst, np.float32)
    W_gcn = np.asarray(W_gcn, np.float32)

    HEADS, HID = att_src.shape
    HC = HEADS * HID
    OUT = W_gcn.shape[1]
    N = x.shape[0]

    pp = _prep(x, edge_index, edge_weight)
    iota, ident = _consts()
    asr = np.tile(att_src.reshape(1, HC), (P, 1)).astype(np.float32)
    adr = np.tile(att_dst.reshape(1, HC), (P, 1)).astype(np.float32)

    nc1 = _build_neff1(pp, HEADS, HID)
    in1 = []
    for c in range(N_CORES):
        in1.append({
            "x": pp["x_pad"], "wg": W_gat, "asr": asr, "adr": adr,
            "iota": iota, "ident": ident,
            "idx16": pp["idx16"][c], "dstloc": pp["dst_s"][c],
            "ew": pp["ew_s"][c], "dstglob": pp["dstg_s"][c],
        })
    res1 = run_bass_kernel_spmd(nc1, in1, core_ids=list(range(N_CORES)),
                                trace=True)
    LAST_EXEC_NS[0] = res1.exec_time_ns

    yfull = np.concatenate([res1.results[c]["yprime"] for c in range(N_CORES)],
                           axis=0).astype(np.float32)
    DEBUG["yfull"] = yfull
    DEBUG["dinv"] = [res1.results[c]["dinv"] for c in range(N_CORES)]
    DEBUG["pp"] = pp

    nc2 = _build_neff2(pp, HEADS, HID, OUT)
    in2 = []
    for c in range(N_CORES):
        in2.append({
            "yfull": yfull, "wgcn": W_gcn, "iota": iota, "ident": ident,
            "idx16": pp["idx16"][c], "dstloc": pp["dst_s"][c],
            "ew": pp["ew_s"][c], "dinv": res1.results[c]["dinv"],
        })
    res2 = run_bass_kernel_spmd(nc2, in2, core_ids=list(range(N_CORES)),
                                trace=True)
    LAST_EXEC_NS[1] = res2.exec_time_ns

    out = np.concatenate([res2.results[c]["outp"] for c in range(N_CORES)],
                         axis=0)[:N]
    return out.astype(np.float32)
